# revision 35
# baseline (speedup 1.0000x reference)
"""LookAheadMask kernel for Trainium2 — in-place, pure-write, paired blocks.

out[b, r, c] = 1.0 if c > r else x[b, r, c], for x of shape (8, 4096, 4096) f32.

Sharding: batch dim across 8 NeuronCores (data parallel, no communication).

The output aliases the input buffer (lowering_input_output_aliases={0: 0}
through the BIR-lowering/NKI path), so everything at/below the diagonal
never moves, and the kernel writes ONLY the strictly-upper triangle:
33.55 MB per core, ZERO reads.

Hardware model distilled from the v1-v11 traces (16 shared DMA engines,
all 8 cores live):
  - The engine pool is byte-bound at ~420-440 GB/s per core; a big
    contiguous descriptor costs ~6 ns + bytes/27.3 per engine. This is
    the binding constraint — the kernel sits within ~10% of it.
  - SBUF source APs: dim 0 must be the partition dim (stride == row
    pitch) and must span all 128 partitions; narrower partition fans
    (64/32) HALVE the per-descriptor rate. Other dims are intra-
    partition offsets. Sub-partition strides in dim 0 read garbage on
    HWDGE (and out-of-range partitions in CoreSim).
  - Each HWDGE dma_start costs ~1.3 us of ring dead time that barely
    pipelines, so bulk work wants few, fat starts (>= 128 descriptors).
  - Small-descriptor DMAs on a HWDGE ring throttle to ~18 ns/desc under
    byte contention and stall that ring's FIFO; fine-grained work
    belongs on the gpsimd SWDGE queue (cheap ~1 us/start issue,
    software desc-gen, coalesces strided rows into 2D descriptors).
    SWDGE semaphore updates don't mix with HWDGE increments on the
    same semaphore — gsem is SWDGE-only. SWDGE is for fine work ONLY:
    routing bulk pairs through it measured +14% (its descriptor
    processing is slow even for contiguous ~9 KiB descs), and deferring
    the 512 B strip descs into the wide-pair window measured +6.7%
    (they steal engine slots from the best-rate descriptors) — both
    reverted; the phase order below is load-bearing.

Decomposition:
  - Bulk: block pairs (2p, 2p+1) quantized to the pair's min width
    w = 3840-256p, one 3D dma_start each ([[S, 128], [DB, 2], [1, w]],
    256 descriptors, source [[3968, 128], [128, 2], [1, w]]). The
    128-wide residual strips of the even blocks all land at constant
    cols [3968, 4096); one 3D start per ring covers 8 of them (strip
    p=15 is block 30's whole bulk rectangle).
  - The strict upper triangles of the 32 diagonal 128x128 blocks:
    binary staircase levels 0/1 (64/32-wide rect sets, one 3D start
    each) plus ragged per-row writes over 32-row groups (one start per
    in-group row m, 128 descriptors of 31-m f32) — all on SWDGE except
    the single-element m=30 group (needs the non-contiguous escape
    hatch SWDGE won't load) which rides at the head of the ACT ring.
  - Three-stage DVE memset of the ones tile ([:, :512], [:, 512:2048],
    rest) phased against pair width so narrow pairs, strips, and all
    fine work start at ~10 us (after the fixed ~8.4 us NEFF preamble)
    while the memset finishes.
"""

import numpy as np

S = 4096
P = 128  # bulk row-block height
NG = S // 32  # 128 ragged row groups
ONES_W = 3968  # widest bulk source row
M0_W = 512  # DVE stage-0; covers fine work, strips, pair 14
M1_W = 2048  # + DVE stage-1; covers pairs 8..13
N_CORES = 8

# Bulk: blocks (2p, 2p+1) merged into one dma_start at the pair's min
# width w = 3840-256p (3D AP, diagonal stride DB between blocks; source
# spans all 128 partitions — narrower partition fans halve the per-desc
# rate). The 128-wide residual strips of the even blocks land at
# constant cols [3968, 4096) — one 3D start per ring covers 8 of them
# (strip p=15 is block 30's whole bulk rectangle).
# Byte-balanced ring split, phased by the memset stage each width needs:
# Pairs 6 and 7 go through the SWDGE queue: a third concurrent
# descriptor stream into the engine pool (the two HWDGE rings alone
# sustain only ~10 of 16 engines).
SP_PAIRS = [[14], [8, 11, 13], [0, 3, 4]]
ACT_PAIRS = [[], [9, 10, 12], [1, 2, 5]]
GP_PAIRS = [7, 6]

_cached = None


def _build():
    from concourse import bass, mybir

    nc = bass.Bass(target_bir_lowering=True, enable_partition_id=False)
    x = nc.dram_tensor("x", [S, S], mybir.dt.float32, kind="ExternalInput")
    out = nc.dram_tensor("out", [S, S], mybir.dt.float32, kind="ExternalOutput")

    DB = P * (S + 1)
    N_HW = 13 + 2 + 1  # HWDGE dma_starts (pairs + strips + ragged m30)
    N_SW = 2 + 2 + 30  # SWDGE dma_starts (L0, L1, pairs 6-7, ragged)

    def pairs(eng, ps, ones, dsem):
        for p in ps:
            w = 3840 - 256 * p
            r0 = 256 * p
            eng.dma_start(
                out=bass.AP(
                    out, r0 * S + r0 + P, [[S, P], [DB, 2], [1, w]]
                ),
                in_=bass.AP(ones, 0, [[ONES_W, P], [P, 2], [1, w]]),
            ).then_inc(dsem, 16)

    def strip(eng, p0, ones, dsem):
        # Residual [128 x 128] squares of the even blocks (+ block 30's
        # whole bulk): rows [256p, 256p+128) x cols [3968, 4096).
        eng.dma_start(
            out=bass.AP(
                out, p0 * 256 * S + ONES_W, [[S, P], [256 * S, 8], [1, P]]
            ),
            in_=bass.AP(ones, 0, [[ONES_W, P], [32, 8], [1, P]]),
        ).then_inc(dsem, 16)

    def level(eng, l, ones, dsem):
        # Binary staircase level l of the strict upper triangles of the 32
        # diagonal 128x128 blocks: n = 32<<l rects of h = 64>>l at stride
        # (128>>l)*(S+1). Partition r of rect k sources ones[r, k*h:k*h+h]
        # (n*h == 2048, inside the first-stage memset).
        s = 128 >> l
        h = s >> 1
        n = 32 << l
        eng.dma_start(
            out=bass.AP(out, h, [[S, h], [s * (S + 1), n], [1, h]]),
            in_=bass.AP(ones, 0, [[ONES_W, h], [256 // n, n], [1, h]]),
        ).then_inc(dsem, 16)

    def ragged(eng, ms, ones, dsem):
        # Row m of each 32-row group covers cols [m+1, 32) of the
        # group-diagonal 32x32 block: 128 descriptors of L = 31-m f32.
        for m in ms:
            L = 31 - m
            with nc.allow_non_contiguous_dma(
                reason="last ragged group writes isolated single f32 cells"
            ):
                eng.dma_start(
                    out=bass.AP(
                        out, 1 + m * (S + 1), [[32 * (S + 1), NG], [1, L]]
                    ),
                    in_=ones[:, :L],
                ).then_inc(dsem, 16)

    with (
        nc.Block() as block,
        nc.semaphore("dsem") as dsem,  # HWDGE output-write DMA completions
        nc.semaphore("gsem") as gsem,  # SWDGE (gpsimd) DMA completions
        nc.semaphore("m0") as m0,  # ones[:, :512] memset done (gpsimd)
        nc.semaphore("m1") as m1,  # + ones[:, 512:2048] done (DVE)
        nc.semaphore("msem") as msem,  # full ones memset done
        nc.sbuf_tensor("ones", [128, ONES_W], mybir.dt.float32) as ones,
    ):

        @block.vector
        def _(vector: bass.BassVectorEngine):
            vector.memset(ones[:, :M0_W], 1.0).then_inc(m0, 1)
            vector.memset(ones[:, M0_W:M1_W], 1.0).then_inc(m1, 1)
            vector.memset(ones[:, M1_W:], 1.0).then_inc(msem, 1)

        @block.sync
        def _(sync: bass.BassEngine):
            sync.wait_ge(m0, 1)
            pairs(sync, SP_PAIRS[0], ones, dsem)
            sync.wait_ge(m1, 1)
            pairs(sync, SP_PAIRS[1][:1], ones, dsem)
            strip(sync, 0, ones, dsem)
            pairs(sync, SP_PAIRS[1][1:], ones, dsem)
            sync.wait_ge(msem, 1)
            pairs(sync, SP_PAIRS[2], ones, dsem)
            sync.wait_ge(dsem, 16 * N_HW)
            sync.wait_ge(gsem, 16 * N_SW)

        @block.scalar
        def _(scalar: bass.BassEngine):
            scalar.wait_ge(m0, 1)
            pairs(scalar, ACT_PAIRS[0], ones, dsem)
            scalar.wait_ge(m1, 1)
            pairs(scalar, ACT_PAIRS[1][:1], ones, dsem)
            ragged(scalar, [30], ones, dsem)
            strip(scalar, 8, ones, dsem)
            pairs(scalar, ACT_PAIRS[1][1:], ones, dsem)
            scalar.wait_ge(msem, 1)
            pairs(scalar, ACT_PAIRS[2], ones, dsem)

        @block.gpsimd
        def _(gpsimd: bass.BassGpSimd):
            gpsimd.wait_ge(m0, 1)
            level(gpsimd, 0, ones, gsem)
            level(gpsimd, 1, ones, gsem)
            gpsimd.wait_ge(msem, 1)
            pairs(gpsimd, GP_PAIRS, ones, gsem)
            ragged(gpsimd, range(30), ones, gsem)

    nc.finalize()
    return nc


def _make_runner():
    """Compile-once runner: jit(shard_map(_body)) over 8 cores with the
    output aliased to the (donated) input — mirrors
    bass2jax.run_bass_via_pjrt, plus lowering_input_output_aliases."""
    global _cached
    if _cached is not None:
        return _cached

    import jax
    from jax.sharding import Mesh, PartitionSpec
    from jax.experimental.shard_map import shard_map
    from concourse import bass2jax

    bass2jax.install_neuronx_cc_hook()
    nc = _build()

    def _body(xg):
        outs = bass2jax._bass_exec_p.bind(
            xg,
            out_avals=(jax.core.ShapedArray((S, S), np.float32),),
            in_names=("x",),
            out_names=("out",),
            lowering_input_output_aliases=((0, 0),),
            sim_require_finite=True,
            sim_require_nnan=True,
            nc=nc,
        )
        return tuple(outs)

    devices = jax.devices()[:N_CORES]
    assert len(devices) == N_CORES, f"need {N_CORES} devices, have {len(devices)}"
    mesh = Mesh(np.asarray(devices), ("core",))
    sharded = jax.jit(
        shard_map(
            _body,
            mesh=mesh,
            in_specs=(PartitionSpec("core"),),
            out_specs=(PartitionSpec("core"),),
            check_rep=False,
        ),
        donate_argnums=(0,),
        keep_unused=True,
    )
    _cached = (nc, sharded)
    return _cached


class _Result:
    def __init__(self, exec_time_ns=None, mean_exec_time_ns=None):
        self.exec_time_ns = exec_time_ns
        self.mean_exec_time_ns = mean_exec_time_ns


def _run(x_full: np.ndarray, trace: bool = False):
    nc, sharded = _make_runner()
    x_full = np.asarray(x_full, dtype=np.float32)
    xg = np.ascontiguousarray(x_full.reshape(N_CORES * S, S))

    if not trace:
        out = sharded(xg)[0]
        return np.asarray(out).reshape(N_CORES, S, S), _Result()

    # Trace path (test.py only): NTFF profile around the execution, then the
    # same gauge/perfetto pipeline run_bass_kernel_spmd uses under axon.
    import glob
    import os
    import tempfile

    from antenv.axon_hooks import get_axon_ntff_profile_hook
    from concourse import bass_utils as BU

    neff_dir = tempfile.mkdtemp()
    hook = get_axon_ntff_profile_hook()
    with hook(neff_dir, [0]):
        out = np.asarray(sharded(xg)[0])

    ntffs = glob.glob(os.path.join(neff_dir, "*_body*.ntff"))
    if not ntffs:
        return out.reshape(N_CORES, S, S), _Result()

    sharepath = BU.upload_artifacts(neff_dir)
    profile = BU.gauge.profiler.Profile(
        profile_path=BU.FishPath(neff_dir),
        kernel_dev_mode=True,
        profile_on_exit=False,
        bass_kernel=nc.m,
        offline_processing=True,
        fname="*_body*",
        annotate_hlo=False,
        metadata={"artifacts_path": sharepath},
    )
    perf = BU._process_ntff_profile(
        profile,
        neff_dir,
        nc,
        list(range(N_CORES)),
        None,
        False,
        {},
        trace_events=False,
    )
    return out.reshape(N_CORES, S, S), _Result(
        perf.exec_time_ns, perf.mean_exec_time_ns
    )


def kernel(x: np.ndarray) -> np.ndarray:
    out, _ = _run(x, trace=False)
    return out
